# revision 12
# baseline (speedup 1.0000x reference)
"""Cosine-similarity kernel (CosineDeconf) for Trainium2, 8 NeuronCores.

out[n, c] = (x[n] . w[c]) / (||x[n]|| * ||w[c]||)
  x: [32768, 512] f32, w: [1000, 512] f32 -> out: [32768, 1000] f32

Strategy (data-parallel over batch, weights replicated):
  - per core: x slab [4096, 512], full w.
  - w prep (once): w_hat = w / ||w||_rows, transposed on PE -> wT [512, 1000].
  - per 128-row batch tile:
      sumsq via DVE tensor_tensor_reduce, rstd = 1/sqrt(sumsq)
      transpose x tile on PE (4x [128,128]) -> xT (f32r)
      matmul (f32r): psum[128, 1000] = xT.T @ wT  (K=512 in 4 chunks)
      evacuate PSUM with per-partition scale rstd on ACT -> SBUF
      DMA -> out
"""

import numpy as np

import concourse.bacc as bacc
import concourse.bass as bass
import concourse.mybir as mybir
import concourse.tile as tile
from concourse.bass_utils import run_bass_kernel_spmd
from concourse.masks import make_identity

N_CORES = 8
BATCH = 32768
D = 512  # in_features
C = 1000  # num_classes
B_CORE = BATCH // N_CORES  # 4096
P = 128
N_BTILES = B_CORE // P  # 32
N_KCH = D // P  # 4
N_CTILES = (C + P - 1) // P  # 8

F32 = mybir.dt.float32
F32R = mybir.dt.float32r

ACT = mybir.ActivationFunctionType
ALU = mybir.AluOpType


def build_bass() -> bass.Bass:
    # Bacc (not Bass): its compile pipeline splits multi-semaphore waits
    # into EventSemaphore instructions, which matmul/ldweights need.
    nc = bacc.Bacc("TRN2")
    x = nc.dram_tensor("x", [B_CORE, D], F32, kind="ExternalInput")
    w = nc.dram_tensor("w", [C, D], F32, kind="ExternalInput")
    out = nc.dram_tensor("out", [B_CORE, C], F32, kind="ExternalOutput")

    with tile.TileContext(nc) as tc:
        with (
            tc.tile_pool(name="singles", bufs=1) as singles,
            tc.tile_pool(name="wstage", bufs=2) as wstage,
            tc.tile_pool(name="xin", bufs=4) as xin,
            tc.tile_pool(name="xTp", bufs=3) as xTp,
            tc.tile_pool(name="stats", bufs=4) as stats,
            tc.tile_pool(name="outs", bufs=3) as outs,
            # separate PSUM pools: w-prep transposes must not contend with
            # the batch loop's transpose/matmul banks
            tc.tile_pool(name="pstw", bufs=1, space="PSUM") as pstwp,
            tc.tile_pool(name="pstx", bufs=3, space="PSUM") as pstxp,
            tc.tile_pool(name="pso", bufs=2, space="PSUM") as psop,
        ):
            ident = singles.tile([P, P], F32)
            make_identity(nc, ident)

            # ---------------- weights prep (once) ----------------
            # wT[p, k, c] = w_hat[c, k*128 + p]
            # f32r: the PSUM->SBUF copy rounds to the matmul input format
            wT = singles.tile([P, N_KCH, C], F32R)
            for j in range(N_CTILES):
                pj = min(P, C - j * P)
                wt = wstage.tile([P, D], F32, tag="wt")
                nc.sync.dma_start(out=wt[:pj], in_=w[j * P : j * P + pj, :])
                wsq = wstage.tile([P, D], F32, tag="wsq")
                wss = stats.tile([P, 1], F32, tag="wss")
                nc.scalar.activation(
                    out=wsq[:pj], in_=wt[:pj], func=ACT.Square, accum_out=wss[:pj]
                )
                wstd = stats.tile([P, 1], F32, tag="wstd")
                nc.scalar.sqrt(wstd[:pj], wss[:pj])
                wrstd = stats.tile([P, 1], F32, tag="wrstd")
                nc.vector.reciprocal(wrstd[:pj], wstd[:pj])
                wh = wstage.tile([P, D], F32, tag="wh")
                nc.scalar.mul(wh[:pj], wt[:pj], wrstd[:pj])
                pst = pstwp.tile([P, 4 * P], F32, tag="pstw")
                for k in range(N_KCH):
                    nc.tensor.transpose(
                        pst[:, k * P : k * P + pj],
                        wh[:pj, k * P : (k + 1) * P],
                        ident[:pj, :pj],
                    )
                src = pst[:, 0 : 4 * P].rearrange("p (k c) -> p k c", k=N_KCH)
                nc.scalar.copy(
                    out=wT[:, :, j * P : j * P + pj], in_=src[:, :, :pj]
                )

            # ---------------- batch loop ----------------
            for i in range(N_BTILES):
                xt = xin.tile([P, D], F32, tag="xt")
                nc.sync.dma_start(out=xt, in_=x[i * P : (i + 1) * P, :])

                xsq = xin.tile([P, D], F32, tag="xsq")
                xss = stats.tile([P, 1], F32, tag="xss")
                nc.scalar.activation(
                    out=xsq, in_=xt, func=ACT.Square, accum_out=xss
                )
                xstd = stats.tile([P, 1], F32, tag="xstd")
                nc.scalar.sqrt(xstd, xss)
                xrstd = stats.tile([P, 1], F32, tag="xrstd")
                nc.vector.reciprocal(xrstd, xstd)

                pst = pstxp.tile([P, 4 * P], F32, tag="pstx")
                for k in range(N_KCH):
                    nc.tensor.transpose(
                        pst[:, k * P : (k + 1) * P],
                        xt[:, k * P : (k + 1) * P],
                        ident,
                    )
                xT = xTp.tile([P, 4 * P], F32R)
                nc.vector.tensor_copy(out=xT, in_=pst)

                po = psop.tile([P, 1024], F32)
                for k in range(N_KCH):
                    nc.tensor.matmul(
                        po[:, 0:512],
                        xT[:, k * P : (k + 1) * P],
                        wT[:, k, 0:512],
                        start=(k == 0),
                        stop=(k == N_KCH - 1),
                    )
                for k in range(N_KCH):
                    nc.tensor.matmul(
                        po[:, 512:1000],
                        xT[:, k * P : (k + 1) * P],
                        wT[:, k, 512:1000],
                        start=(k == 0),
                        stop=(k == N_KCH - 1),
                    )

                ob = outs.tile([P, C], F32)
                # evacuate PSUM with the 1/||x|| row scale fused in,
                # split across ScalarE and VectorE
                nc.scalar.mul(ob[:, 0:512], po[:, 0:512], xrstd)
                nc.vector.tensor_scalar_mul(ob[:, 512:1000], po[:, 512:1000], xrstd)
                nc.sync.dma_start(out=out[i * P : (i + 1) * P, :], in_=ob)

    nc.finalize()  # runs Bacc's compile passes (alloc_regs, wait splitting)
    return nc


_NC_CACHE = None


def _get_nc():
    global _NC_CACHE
    if _NC_CACHE is None:
        _NC_CACHE = build_bass()
    return _NC_CACHE


def run(x, weights, **spmd_kwargs):
    x = np.ascontiguousarray(np.asarray(x, dtype=np.float32))
    weights = np.ascontiguousarray(np.asarray(weights, dtype=np.float32))
    assert x.shape == (BATCH, D), x.shape
    assert weights.shape == (C, D), weights.shape
    nc = _get_nc()
    in_maps = [
        {"x": x[i * B_CORE : (i + 1) * B_CORE], "w": weights}
        for i in range(N_CORES)
    ]
    res = run_bass_kernel_spmd(nc, in_maps, core_ids=list(range(N_CORES)), **spmd_kwargs)
    out = np.concatenate([r["out"] for r in res.results], axis=0)
    return out, res


def kernel(x, weights):
    out, _ = run(x, weights)
    return out


# revision 17
# speedup vs baseline: 1.1662x; 1.1662x over previous
"""Cosine-similarity kernel (CosineDeconf) for Trainium2, 8 NeuronCores.

out[n, c] = (x[n] . w[c]) / (||x[n]|| * ||w[c]||)
  x: [32768, 512] f32, w: [1000, 512] f32 -> out: [32768, 1000] f32

Strategy (data-parallel over batch, weights replicated):
  - per core: x slab [4096, 512], full w.
  - w prep (once): w_hat = w / ||w||_rows, transposed on PE -> wT [512, 1000].
  - per 128-row batch tile (in groups of 4 sharing one sqrt/reciprocal):
      sumsq on GpSimd (mult+reduce)
      transpose x tile on PE (4x [128,128]) -> xT (f32r via DVE copy)
      matmul (f32r): psum[128, 1000] = xT.T @ wT  (K=512 in 4 chunks)
      evacuate PSUM with the 1/||x|| scale fused (ACT half, DVE half)
      DMA 2 tiles at a time -> out
"""

import numpy as np

import concourse.bacc as bacc
import concourse.bass as bass
import concourse.mybir as mybir
import concourse.tile as tile
from concourse.bass_utils import run_bass_kernel_spmd
from concourse.masks import make_identity

N_CORES = 8
BATCH = 32768
D = 512  # in_features
C = 1000  # num_classes
B_CORE = BATCH // N_CORES  # 4096
P = 128
N_BTILES = B_CORE // P  # 32
N_KCH = D // P  # 4
N_CTILES = (C + P - 1) // P  # 8
SGRP = 4  # batch tiles per stats group (shared sqrt/reciprocal)
OGRP = 2  # batch tiles per output DMA

F32 = mybir.dt.float32
F32R = mybir.dt.float32r

ACT = mybir.ActivationFunctionType
ALU = mybir.AluOpType


def build_bass() -> bass.Bass:
    # Bacc (not Bass): its compile pipeline splits multi-semaphore waits
    # into EventSemaphore instructions, which matmul/ldweights need.
    nc = bacc.Bacc("TRN2")
    x = nc.dram_tensor("x", [B_CORE, D], F32, kind="ExternalInput")
    w = nc.dram_tensor("w", [C, D], F32, kind="ExternalInput")
    out = nc.dram_tensor("out", [B_CORE, C], F32, kind="ExternalOutput")

    with tile.TileContext(nc) as tc:
        with (
            tc.tile_pool(name="singles", bufs=1) as singles,
            tc.tile_pool(name="wstage", bufs=2) as wstage,
            tc.tile_pool(name="xin", bufs=10) as xin,
            tc.tile_pool(name="xsqp", bufs=3) as xsqp,
            tc.tile_pool(name="xTp", bufs=8) as xTp,
            tc.tile_pool(name="stats", bufs=3) as stats,
            tc.tile_pool(name="outs", bufs=3) as outs,
            # separate PSUM pools: w-prep transposes must not contend with
            # the batch loop's transpose/matmul banks
            tc.tile_pool(name="pstw", bufs=1, space="PSUM") as pstwp,
            tc.tile_pool(name="pstx", bufs=3, space="PSUM") as pstxp,
            tc.tile_pool(name="pso", bufs=2, space="PSUM") as psop,
        ):
            ident = singles.tile([P, P], F32)
            make_identity(nc, ident)

            # ---------------- weights prep (once) ----------------
            # wT[p, k, c] = w_hat[c, k*128 + p]
            # f32r: the PSUM->SBUF copy rounds to the matmul input format
            wT = singles.tile([P, N_KCH, C], F32R)
            wss = singles.tile([P, N_CTILES], F32)
            nc.vector.memset(wss, 1.0)
            wts = []
            for j in range(N_CTILES):
                pj = min(P, C - j * P)
                wt = wstage.tile([P, D], F32, tag=f"wt{j}")
                wts.append(wt)
                nc.sync.dma_start(out=wt[:pj], in_=w[j * P : j * P + pj, :])
                wsq = wstage.tile([P, D], F32, tag="wsq")
                nc.scalar.activation(
                    out=wsq[:pj],
                    in_=wt[:pj],
                    func=ACT.Square,
                    accum_out=wss[:pj, j : j + 1],
                )
            wstd = singles.tile([P, N_CTILES], F32)
            nc.scalar.sqrt(wstd, wss)
            wrstd = singles.tile([P, N_CTILES], F32)
            nc.vector.reciprocal(wrstd, wstd)
            for j in range(N_CTILES):
                pj = min(P, C - j * P)
                wh = wstage.tile([P, D], F32, tag="wh")
                nc.vector.tensor_scalar_mul(
                    wh[:pj], wts[j][:pj], wrstd[:pj, j : j + 1]
                )
                pst = pstwp.tile([P, 4 * P], F32, tag="pstw")
                for k in range(N_KCH):
                    nc.tensor.transpose(
                        pst[:, k * P : k * P + pj],
                        wh[:pj, k * P : (k + 1) * P],
                        ident[:pj, :pj],
                    )
                src = pst[:, 0 : 4 * P].rearrange("p (k c) -> p k c", k=N_KCH)
                nc.scalar.copy(
                    out=wT[:, :, j * P : j * P + pj], in_=src[:, :, :pj]
                )

            # ---------------- batch loop ----------------
            for g in range(N_BTILES // SGRP):
                xts = []
                xss = stats.tile([P, SGRP], F32, tag="xss")
                for t in range(SGRP):
                    i = g * SGRP + t
                    xt = xin.tile([P, D], F32, tag="xt")
                    xts.append(xt)
                    nc.sync.dma_start(out=xt, in_=x[i * P : (i + 1) * P, :])
                    # square on GpSimd (otherwise idle engine), reduce on DVE
                    xsq = xsqp.tile([P, D], F32, tag="xsq")
                    nc.gpsimd.tensor_mul(xsq, xt, xt)
                    nc.vector.reduce_sum(
                        xss[:, t : t + 1], xsq, axis=mybir.AxisListType.X
                    )
                xstd = stats.tile([P, SGRP], F32, tag="xstd")
                nc.scalar.sqrt(xstd, xss)
                xrstd = stats.tile([P, SGRP], F32, tag="xrstd")
                nc.vector.reciprocal(xrstd, xstd)

                ob = None
                for t in range(SGRP):
                    i = g * SGRP + t
                    xt = xts[t]
                    pst = pstxp.tile([P, 4 * P], F32, tag="pstx")
                    for k in range(N_KCH):
                        nc.tensor.transpose(
                            pst[:, k * P : (k + 1) * P],
                            xt[:, k * P : (k + 1) * P],
                            ident,
                        )
                    xT = xTp.tile([P, 4 * P], F32R)
                    nc.vector.tensor_copy(out=xT, in_=pst)

                    po = psop.tile([P, 1024], F32)
                    for k in range(N_KCH):
                        nc.tensor.matmul(
                            po[:, 0:512],
                            xT[:, k * P : (k + 1) * P],
                            wT[:, k, 0:512],
                            start=(k == 0),
                            stop=(k == N_KCH - 1),
                            skip_group_check=True,
                        )
                        nc.tensor.matmul(
                            po[:, 512:1000],
                            xT[:, k * P : (k + 1) * P],
                            wT[:, k, 512:1000],
                            start=(k == 0),
                            stop=(k == N_KCH - 1),
                            skip_group_check=True,
                        )

                    if t % OGRP == 0:
                        ob = outs.tile([P, OGRP, C], F32, tag="ob")
                    u = t % OGRP
                    sc = xrstd[:, t : t + 1]
                    # evacuate PSUM with the 1/||x|| row scale fused in (ACT)
                    nc.scalar.mul(ob[:, u, 0:512], po[:, 0:512], sc)
                    nc.scalar.mul(ob[:, u, 512:1000], po[:, 512:1000], sc)
                    if u == OGRP - 1:
                        i0 = i - (OGRP - 1)
                        dst = out[i0 * P : (i0 + OGRP) * P, :].rearrange(
                            "(u p) c -> p u c", p=P
                        )
                        nc.sync.dma_start(out=dst, in_=ob)

    nc.finalize()  # runs Bacc's compile passes (alloc_regs, wait splitting)
    return nc


_NC_CACHE = None


def _get_nc():
    global _NC_CACHE
    if _NC_CACHE is None:
        _NC_CACHE = build_bass()
    return _NC_CACHE


def run(x, weights, **spmd_kwargs):
    x = np.ascontiguousarray(np.asarray(x, dtype=np.float32))
    weights = np.ascontiguousarray(np.asarray(weights, dtype=np.float32))
    assert x.shape == (BATCH, D), x.shape
    assert weights.shape == (C, D), weights.shape
    nc = _get_nc()
    in_maps = [
        {"x": x[i * B_CORE : (i + 1) * B_CORE], "w": weights}
        for i in range(N_CORES)
    ]
    res = run_bass_kernel_spmd(nc, in_maps, core_ids=list(range(N_CORES)), **spmd_kwargs)
    out = np.concatenate([r["out"] for r in res.results], axis=0)
    return out, res


def kernel(x, weights):
    out, _ = run(x, weights)
    return out
